# revision 33
# baseline (speedup 1.0000x reference)
"""Fused CNN-LSTM cell (locked dropout) Trainium2 kernel.

Math (per row b of a batch of B):
    concat = [x_t, h_prev] * mask[b]          # [B, 128]
    gates  = concat @ [W_i|W_f|W_o|W_g] + b   # [B, 256]
    i,f,o  = sigmoid(gates[:, :192]);  g = tanh(gates[:, 192:])
    c      = f * c_prev + i * g
    h      = o * tanh(c)
    returns (h, c)

Distribution: data-parallel over the batch dim across 8 NeuronCores
(32768 rows/core); gate weights replicated.

Per-core dataflow (macro = 8 chunks of 128 rows):
    - gpsimd (SWDGE) cast-DMA loads x/h/c_prev fp32->bf16, batch-major
      [128 part = row-in-chunk, chunk, feat]
    - DVE tensor_scalar multiplies the concat tile by the per-row dropout
      mask (chunk-major mask resident in SBUF)
    - xbar DMA-transpose (HWDGE) turns each [row, feat] 128x128 block into
      [feat, row] so the feature dim lands on partitions for the matmul
    - PE: gates[128,256] = catT.T @ W (bf16, fp32 psum), bias added with a
      K=1 accumulating matmul (ones[1,128].T @ b[1,512] over chunk pairs)
    - ACT: sigmoid over the i|f|o columns, tanh over g, tanh(c)
    - DVE: f*c_prev, i*g, add, o*tanh(c) in bf16 (2x mode)
    - gpsimd cast-DMA stores bf16->fp32 straight to DRAM
"""

import numpy as np

from concourse import bacc, masks, mybir, tile
from concourse.bass_utils import run_bass_kernel_spmd

B, D, H = 262144, 64, 64
N_CORES = 8
B_LOC = B // N_CORES  # 32768
CHUNK = 128           # rows per matmul tile (partition dim)
MACRO = 16            # chunks per macro-iteration
HALF = MACRO // 2     # chunks per transpose batch
QUART = MACRO // 4    # chunks per PSUM group (2 banks)

F32 = mybir.dt.float32
BF16 = mybir.dt.bfloat16
GATE_ORDER = ("i", "f", "o", "g")


def build_bass(b_loc: int = B_LOC, load_bufs: int = 5, work_bufs: int = 4,
               n_transp: int = 1, gate_group: int = 8, psum_bufs: int = 2,
               prefetch: int = 2):
    assert b_loc % (CHUNK * MACRO) == 0
    n_chunks = b_loc // CHUNK
    assert n_chunks % 16 == 0  # xbar-transpose partition granularity for the mask
    n_macro = b_loc // (CHUNK * MACRO)

    nc = bacc.Bacc("TRN2", target_bir_lowering=False, debug=False)

    x_d = nc.dram_tensor("x_t", [b_loc, D], F32, kind="ExternalInput")
    h_d = nc.dram_tensor("h_prev", [b_loc, H], F32, kind="ExternalInput")
    c_d = nc.dram_tensor("c_prev", [b_loc, H], F32, kind="ExternalInput")
    m_d = nc.dram_tensor("mask", [b_loc, 1], F32, kind="ExternalInput")
    w_d = {g: nc.dram_tensor(f"W_{g}", [D + H, H], F32, kind="ExternalInput")
           for g in GATE_ORDER}
    b_d = {g: nc.dram_tensor(f"b_{g}", [1, H], F32, kind="ExternalInput")
           for g in GATE_ORDER}
    ho_d = nc.dram_tensor("h_out", [b_loc, H], F32, kind="ExternalOutput")
    co_d = nc.dram_tensor("c_out", [b_loc, H], F32, kind="ExternalOutput")

    # batch-major chunked views: [128 rows-in-chunk, n_chunks, feat]
    xv = x_d[:].rearrange("(k p) f -> p k f", p=CHUNK)
    hv = h_d[:].rearrange("(k p) f -> p k f", p=CHUNK)
    cv = c_d[:].rearrange("(k p) f -> p k f", p=CHUNK)
    hov = ho_d[:].rearrange("(k p) f -> p k f", p=CHUNK)
    cov = co_d[:].rearrange("(k p) f -> p k f", p=CHUNK)
    # mask grouped by chunk: [n_chunks, 128]
    mv = m_d[:].rearrange("(k p) one -> k (p one)", p=CHUNK)

    with tile.TileContext(nc) as tc:
        with tc.tile_pool(name="const", bufs=1) as constp, \
             tc.tile_pool(name="loads", bufs=load_bufs) as loadp, \
             tc.tile_pool(name="work", bufs=work_bufs) as workp:

            # ---- one-time constants ----
            w_bf = constp.tile([D + H, 4 * H], BF16)     # [128, 256]
            b2_bf = constp.tile([1, 2 * 4 * H], BF16)    # bias repeated twice: [1, 512]
            ones_bf = constp.tile([1, CHUNK], BF16)
            mask_cm = constp.tile([CHUNK, n_chunks], F32)  # chunk-major dropout mask
            identity = constp.tile([CHUNK, CHUNK], F32)

            for gi, g in enumerate(GATE_ORDER):
                nc.gpsimd.dma_start(w_bf[:, gi * H:(gi + 1) * H], w_d[g][:])
                for rep in range(2):
                    nc.gpsimd.dma_start(
                        b2_bf[:, rep * 4 * H + gi * H: rep * 4 * H + (gi + 1) * H],
                        b_d[g][:])
            nc.vector.memset(ones_bf[:], 1.0)
            masks.make_identity(nc, identity[:])

            # mask: load [chunk, row] groups, PE-transpose to chunk-major [row, chunk]
            with tc.tile_pool(name="mpsum", bufs=1, space="PSUM") as mpsump:
                for g0 in range(0, n_chunks, CHUNK):
                    gsz = min(CHUNK, n_chunks - g0)
                    mtmp = loadp.tile([gsz, CHUNK], F32, tag="mask_tmp")
                    nc.sync.dma_start(mtmp[:], mv[g0:g0 + gsz, :])
                    mps = mpsump.tile([CHUNK, CHUNK], F32, tag="mask_ps")
                    nc.tensor.transpose(mps[:, 0:gsz], mtmp[:], identity[0:gsz, 0:gsz])
                    nc.vector.tensor_copy(mask_cm[:, g0:g0 + gsz], mps[:, 0:gsz])

            # ---- main loop, software-pipelined over macros ----
            # Engine-stream discipline (each DMA-issuing sequencer is
            # in-order, so a stream must never mix late-stage waits ahead of
            # early-stage work):
            #   sync (SP):    x/h prefetch loads + half the transposes
            #   scalar (ACT): other half of the transposes (+ activations)
            #   gpsimd (Pool): c_prev prefetch + cast-stores (late stage)
            # stage A(m): mask-mul, transpose, matmuls, sigmoid/tanh
            # stage B(m): elementwise LSTM tail + stores — emitted one macro
            # later so the ACT and DVE instruction streams never ping-pong.
            PREFETCH = prefetch
            psump = tc.alloc_tile_pool(name="psum", bufs=2, space="PSUM")
            stash = {}
            loaded = {}

            def issue_loads(m):
                # all prefetch loads on gpsimd/SWDGE with fp32->bf16 cast:
                # the Pool stream reads only DRAM (never waits), so it can
                # run arbitrarily far ahead
                ks = slice(m * MACRO, (m + 1) * MACRO)
                catm = loadp.tile([CHUNK, MACRO, D + H], BF16, tag="catm")
                nc.gpsimd.dma_start(catm[:, :, 0:D], xv[:, ks, :])
                nc.gpsimd.dma_start(catm[:, :, D:D + H], hv[:, ks, :])
                # cpb lives from prefetch until stage_b two macros later
                cpb = loadp.tile([CHUNK, MACRO, H], BF16, tag="cpb", bufs=7)
                nc.gpsimd.dma_start(cpb[:], cv[:, ks, :])
                loaded[m] = (catm, cpb)

            def stage_a(m):
                catm, cpb = loaded.pop(m)
                # mask-mul, then transposes in n_transp batches; matmuls in
                # gate_group-chunk PSUM groups (gate_group/2 banks each)
                catms = workp.tile([CHUNK, MACRO, D + H], BF16, tag="catms")
                catT = workp.tile([D + H, MACRO, CHUNK], BF16, tag="catT")
                gates_groups = []
                tspan = MACRO // n_transp
                for h in range(n_transp):
                    for k in range(h * tspan, (h + 1) * tspan):
                        kk = m * MACRO + k
                        nc.vector.tensor_scalar_mul(
                            catms[:, k, :], catm[:, k, :], mask_cm[:, kk:kk + 1])
                    hs = slice(h * tspan, (h + 1) * tspan)
                    nc.sync.dma_start_transpose(catT[:, hs, :], catms[:, hs, :])
                    # one accumulation group per 2KB PSUM bank (2 chunks/bank):
                    # start=True clears has_written bank-wide, so it appears
                    # exactly once per bank, before everything else in it
                    for g in range(tspan // gate_group):
                        gates = psump.tile([CHUNK, gate_group, 4 * H], F32,
                                           tag="gates", bufs=psum_bufs)
                        for kb in range(gate_group // 2):
                            k0 = h * tspan + g * gate_group + 2 * kb
                            nc.tensor.matmul(gates[:, 2 * kb, :], catT[:, k0, :],
                                             w_bf[:], start=True, stop=False)
                            nc.tensor.matmul(gates[:, 2 * kb + 1, :],
                                             catT[:, k0 + 1, :],
                                             w_bf[:], start=False, stop=False)
                            nc.tensor.matmul(
                                gates[:, 2 * kb:2 * kb + 2, :].rearrange(
                                    "p a b -> p (a b)"),
                                ones_bf[:], b2_bf[:],
                                start=False, stop=True, skip_group_check=True)
                        gates_groups.append(gates)

                stash[m] = (gates_groups, cpb)
                return None

            def stage_act(m):
                gates_groups, cpb = stash.pop(m)
                ifo = workp.tile([CHUNK, MACRO, 3 * H], BF16, tag="ifo")
                gt = workp.tile([CHUNK, MACRO, H], BF16, tag="gt")
                gg = MACRO // len(gates_groups)
                for q, gates in enumerate(gates_groups):
                    qs = slice(q * gg, (q + 1) * gg)
                    nc.scalar.activation(ifo[:, qs, :], gates[:, :, 0:3 * H],
                                         mybir.ActivationFunctionType.Sigmoid)
                    nc.scalar.activation(gt[:, qs, :], gates[:, :, 3 * H:4 * H],
                                         mybir.ActivationFunctionType.Tanh)
                stash[m] = (ifo, gt, cpb)

            def stage_b1(m):
                # c = f*c_prev + i*g on DVE, then tanh(c) on ACT
                ifo, gt, cpb = stash.pop(m)
                t1 = workp.tile([CHUNK, MACRO, H], BF16, tag="t1")
                t2 = workp.tile([CHUNK, MACRO, H], BF16, tag="t2")
                cb = workp.tile([CHUNK, MACRO, H], BF16, tag="cb", bufs=5)
                th = workp.tile([CHUNK, MACRO, H], BF16, tag="th", bufs=5)
                nc.vector.tensor_mul(t1[:], ifo[:, :, H:2 * H], cpb[:])   # f * c_prev
                nc.vector.tensor_mul(t2[:], ifo[:, :, 0:H], gt[:])        # i * g
                nc.vector.tensor_add(cb[:], t1[:], t2[:])                 # c
                nc.scalar.activation(th[:], cb[:],
                                     mybir.ActivationFunctionType.Tanh)
                stash[("b", m)] = (ifo, cb, th)

            def stage_b2(m):
                # widen to fp32 on DVE, store via HWDGE (SP) — keeps the Pool
                # stream (prefetch loads) free of late-stage waits
                ks = slice(m * MACRO, (m + 1) * MACRO)
                ifo, cb, th = stash.pop(("b", m))
                cf = workp.tile([CHUNK, MACRO, H], F32, tag="cf")
                hf = workp.tile([CHUNK, MACRO, H], F32, tag="hf")
                nc.vector.tensor_copy(cf[:], cb[:])
                nc.vector.tensor_mul(hf[:], ifo[:, :, 2 * H:3 * H], th[:])
                nc.sync.dma_start(cov[:, ks, :], cf[:])
                nc.sync.dma_start(hov[:, ks, :], hf[:])

            for m in range(min(PREFETCH, n_macro)):
                issue_loads(m)
            for m in range(n_macro + 3):
                if m + PREFETCH < n_macro:
                    issue_loads(m + PREFETCH)
                if m < n_macro:
                    stage_a(m)
                if 1 <= m <= n_macro:
                    stage_act(m - 1)
                if 2 <= m <= n_macro + 1:
                    stage_b1(m - 2)
                if m >= 3:
                    stage_b2(m - 3)

            psump.release()

    nc.compile()
    return nc


_CACHED_NC = None


def _get_nc():
    global _CACHED_NC
    if _CACHED_NC is None:
        _CACHED_NC = build_bass(B_LOC)
    return _CACHED_NC


def make_in_maps(inputs: dict, b_loc: int = B_LOC, n_cores: int = N_CORES):
    in_maps = []
    for c in range(n_cores):
        sl = slice(c * b_loc, (c + 1) * b_loc)
        im = {
            "x_t": np.ascontiguousarray(inputs["x_t"][sl], dtype=np.float32),
            "h_prev": np.ascontiguousarray(inputs["h_prev"][sl], dtype=np.float32),
            "c_prev": np.ascontiguousarray(inputs["c_prev"][sl], dtype=np.float32),
            "mask": np.ascontiguousarray(inputs["mask"][sl], dtype=np.float32),
        }
        for g in GATE_ORDER:
            im[f"W_{g}"] = np.ascontiguousarray(inputs[f"W_{g}"], dtype=np.float32)
            im[f"b_{g}"] = np.ascontiguousarray(
                np.asarray(inputs[f"b_{g}"], dtype=np.float32).reshape(1, H))
        in_maps.append(im)
    return in_maps


def kernel(**inputs):
    nc = _get_nc()
    in_maps = make_in_maps(inputs)
    res = run_bass_kernel_spmd(nc, in_maps, core_ids=list(range(N_CORES)))
    h = np.concatenate([res.results[c]["h_out"] for c in range(N_CORES)], axis=0)
    c = np.concatenate([res.results[c]["c_out"] for c in range(N_CORES)], axis=0)
    return (h, c)
